# revision 4
# baseline (speedup 1.0000x reference)
"""Barycentric pooling (segmented Sinkhorn onto a 16x256 codebook) on 8 trn2 cores.

Strategy (data-parallel per sharding hint): host sorts nodes by graph, pads each
graph to a fixed node count, and assigns 16 graphs to each of the 8 cores.  Each
core gets a hidden-major [258, NPTS] tensor (256 hidden rows + squared-norm row
+ ones row).  On device, per 128-point chunk, three accumulating matmuls against
the (augmented) codebook produce -cost/2 in PSUM; ScalarE exp(20*x) turns that
into K = exp(-cost/eps) stored point-major [128, NPAD, 16] in SBUF.  Graph g of
a core owns partitions [8g, 8g+8).  The 20 Sinkhorn iterations run on-chip:
VectorE multiply+reduce for the two K passes, TensorE for the tiny
cross-partition sums (graph-sum via a selection matrix, v-broadcast rebuild).
Cores are fully independent; the host assembles the [128, 16] output.
"""

import numpy as np

import concourse.bass as bass
import concourse.bacc as bacc
import concourse.mybir as mybir
from concourse import tile
from concourse.bass_utils import run_bass_kernel_spmd

B = 128          # graphs
CB = 16          # codebook size
HID = 256
DIST = 8
EPS = 0.1
ITERS = 20
NCORES = 8
GPC = B // NCORES  # graphs per core = 16

F32 = mybir.dt.float32


def build_nc(NPAD: int, TQ: int = 8):
    """Build the single-core SPMD program.  NPAD = padded nodes per graph
    (so NPAD points per partition, 128*NPAD points per core)."""
    NPTS = 128 * NPAD
    n_chunks = NPAD              # 128-point chunks (chunk c = q-slot c)
    assert n_chunks % TQ == 0
    n_stages = n_chunks // TQ

    nc = bacc.Bacc(None, target_bir_lowering=False, debug=False)

    xt_ext = nc.declare_dram_parameter("xt", [HID, NPTS], F32, isOutput=False)
    aug_ext = nc.declare_dram_parameter("aug", [2, NPTS], F32, isOutput=False)
    wm_ext = nc.declare_dram_parameter("wmat", [HID, CB], F32, isOutput=False)
    wa_ext = nc.declare_dram_parameter("waug", [2, CB], F32, isOutput=False)
    asc_ext = nc.declare_dram_parameter("asc", [CB, 1], F32, isOutput=False)
    av_ext = nc.declare_dram_parameter("av", [CB, 1], F32, isOutput=False)
    sel_ext = nc.declare_dram_parameter("sel", [128, CB], F32, isOutput=False)
    selt_ext = nc.declare_dram_parameter("selt", [CB, 128], F32, isOutput=False)
    out_ext = nc.declare_dram_parameter("out", [CB, CB], F32, isOutput=True)

    with tile.TileContext(nc) as tc:
        with (
            tc.tile_pool(name="const", bufs=1) as cpool,
            tc.tile_pool(name="stage", bufs=3) as spool,
            tc.tile_pool(name="work", bufs=2) as wpool,
            tc.tile_pool(name="small", bufs=2) as mpool,
            tc.tile_pool(name="acc", bufs=3, space=bass.MemorySpace.PSUM) as apool,
            tc.tile_pool(name="psmall", bufs=2, space=bass.MemorySpace.PSUM) as ppool,
        ):
            # ---- constants ----
            wm_sb = cpool.tile([128, 2, CB], F32, tag="wm")
            nc.sync.dma_start(wm_sb[:, 0, :], wm_ext[0:128, :])
            nc.sync.dma_start(wm_sb[:, 1, :], wm_ext[128:256, :])
            wa_sb = cpool.tile([2, CB], F32, tag="wa")
            nc.sync.dma_start(wa_sb[:], wa_ext[:, :])
            sel_sb = cpool.tile([128, CB], F32, tag="sel")
            nc.sync.dma_start(sel_sb[:], sel_ext[:, :])
            selt_sb = cpool.tile([CB, 128], F32, tag="selt")
            nc.sync.dma_start(selt_sb[:], selt_ext[:, :])
            asc_sb = cpool.tile([CB, 1], F32, tag="asc")
            nc.sync.dma_start(asc_sb[:], asc_ext[:, :])
            av_sb = cpool.tile([CB, 1], F32, tag="av")
            nc.sync.dma_start(av_sb[:], av_ext[:, :])

            k_sb = cpool.tile([128, NPAD, CB], F32, tag="K")
            v_sb = cpool.tile([CB, CB], F32, tag="v")
            nc.vector.memset(v_sb[:], 1.0)

            # ---- phase 1: K = exp(-cost/eps), point-major ----
            for st in range(n_stages):
                c0 = st * TQ
                cols = slice(c0 * 128, (c0 + TQ) * 128)
                xa0 = spool.tile([128, TQ * 128], F32, tag="xa0")
                xa1 = spool.tile([128, TQ * 128], F32, tag="xa1")
                xag = spool.tile([2, TQ * 128], F32, tag="xag")
                nc.sync.dma_start(xa0[:], xt_ext[0:128, cols])
                nc.sync.dma_start(xa1[:], xt_ext[128:256, cols])
                nc.sync.dma_start(xag[:], aug_ext[:, cols])
                for cl in range(TQ):
                    c = c0 + cl
                    pts = slice(cl * 128, (cl + 1) * 128)
                    acc = apool.tile([128, CB], F32, tag="acc")
                    nc.tensor.matmul(acc[:], xa0[:, pts], wm_sb[:, 0, :],
                                     start=True, stop=False)
                    nc.tensor.matmul(acc[:], xa1[:, pts], wm_sb[:, 1, :],
                                     start=False, stop=False)
                    nc.tensor.matmul(acc[:], xag[:, pts], wa_sb[:],
                                     start=False, stop=True)
                    # K = exp(20 * (-cost/2)) = exp(-cost/0.1)
                    nc.scalar.activation(k_sb[:, c, :], acc[:],
                                         mybir.ActivationFunctionType.Exp,
                                         scale=20.0)

            # ---- phase 2: 20 Sinkhorn iterations ----
            for it in range(ITERS):
                # vrep[p, j] = v[p//8, j]
                vrep = ppool.tile([128, CB], F32, tag="vrep")
                nc.tensor.matmul(vrep[:], selt_sb[:], v_sb[:],
                                 start=True, stop=True)
                # d[p, q] = sum_j K[p,q,j] * v[g(p), j]
                prod = wpool.tile([128, NPAD, CB], F32, tag="prod")
                nc.vector.tensor_tensor(
                    prod[:], k_sb[:],
                    vrep[:].unsqueeze(1).broadcast_to([128, NPAD, CB]),
                    mybir.AluOpType.mult)
                d_sb = mpool.tile([128, NPAD], F32, tag="d")
                nc.vector.tensor_reduce(d_sb[:], prod[:],
                                        mybir.AxisListType.X,
                                        mybir.AluOpType.add)
                # w = 1 / (d + 1e-8)
                nc.vector.tensor_scalar_add(d_sb[:], d_sb[:], 1e-8)
                w_sb = mpool.tile([128, NPAD], F32, tag="w")
                nc.vector.reciprocal(w_sb[:], d_sb[:])
                # sp[p, j] = sum_q K[p,q,j] * w[p,q]   (write j-major, reduce X)
                prods = wpool.tile([128, CB, NPAD], F32, tag="prod")
                nc.vector.tensor_tensor(
                    prods[:].transpose([0, 2, 1]), k_sb[:],
                    w_sb[:].unsqueeze(2).broadcast_to([128, NPAD, CB]),
                    mybir.AluOpType.mult)
                sp_sb = mpool.tile([128, CB], F32, tag="sp")
                nc.vector.tensor_reduce(sp_sb[:], prods[:],
                                        mybir.AxisListType.X,
                                        mybir.AluOpType.add)
                # s_raw[k, j] = sum_{p in graph k} sp[p, j]
                sraw = ppool.tile([CB, CB], F32, tag="sraw")
                nc.tensor.matmul(sraw[:], sel_sb[:], sp_sb[:],
                                 start=True, stop=True)
                if it < ITERS - 1:
                    # v = 1 / (16*a*s_raw + 16e-8)
                    vden = mpool.tile([CB, CB], F32, tag="vden")
                    nc.vector.tensor_scalar(vden[:], sraw[:], asc_sb[:], 1.6e-7,
                                            mybir.AluOpType.mult,
                                            mybir.AluOpType.add)
                    nc.vector.reciprocal(v_sb[:], vden[:])
                else:
                    # final: tsum = a*s_raw*v ; weights = tsum / max(sum_j tsum, tiny)
                    t1 = mpool.tile([CB, CB], F32, tag="vden")
                    nc.vector.tensor_scalar(t1[:], sraw[:], av_sb[:], None,
                                            mybir.AluOpType.mult)
                    t2 = mpool.tile([CB, CB], F32, tag="t2")
                    nc.vector.tensor_tensor(t2[:], t1[:], v_sb[:],
                                            mybir.AluOpType.mult)
                    den = mpool.tile([CB, 1], F32, tag="den")
                    nc.vector.tensor_reduce(den[:], t2[:],
                                            mybir.AxisListType.X,
                                            mybir.AluOpType.add)
                    nc.vector.tensor_scalar(den[:], den[:], 1e-30, None,
                                            mybir.AluOpType.max)
                    rden = mpool.tile([CB, 1], F32, tag="rden")
                    nc.vector.reciprocal(rden[:], den[:])
                    outw = mpool.tile([CB, CB], F32, tag="outw")
                    nc.vector.tensor_scalar(outw[:], t2[:], rden[:], None,
                                            mybir.AluOpType.mult)
                    nc.sync.dma_start(out_ext[:, :], outw[:])

    return nc


def _host_shard(node_distributions, batch_idx, codebook):
    nd = np.ascontiguousarray(np.asarray(node_distributions, dtype=np.float32))
    bi = np.asarray(batch_idx).astype(np.int64).ravel()
    cb = np.asarray(codebook, dtype=np.float32)
    N, S, D = nd.shape
    assert S == DIST and D == HID

    counts = np.bincount(bi, minlength=B)[:B]
    NPAD = int(np.ceil(max(int(counts.max()), 64) / 64.0) * 64)
    NPTS = 128 * NPAD

    order = np.argsort(bi, kind="stable")
    slot = np.full((B, NPAD), -1, dtype=np.int64)
    mask = np.arange(NPAD)[None, :] < counts[:, None]
    slot[mask] = order

    # column t of a core's xt: p = t%128, q = t//128; graph slot k = p//8;
    # m = (p%8)*NPAD + q; node_local = m//S; s = m%S
    t = np.arange(NPTS)
    p = t % 128
    q = t // 128
    k_of_t = p // 8
    m = (p % 8) * NPAD + q
    nl_of_t = m // S
    s_of_t = m % S

    yn = (cb * cb).sum(1)
    wmat = np.ascontiguousarray(cb.T)                      # [256, 16]
    waug = np.stack([np.full(CB, -0.5, np.float32),
                     (-0.5 * yn).astype(np.float32)])      # [2, 16]
    sel = np.zeros((128, CB), np.float32)
    sel[np.arange(128), np.arange(128) // 8] = 1.0
    selt = np.ascontiguousarray(sel.T)

    in_maps = []
    for c in range(NCORES):
        g = c * GPC + k_of_t                    # global graph per column
        nid = slot[g, nl_of_t]                  # node id or -1
        valid = nid >= 0
        x = nd[np.where(valid, nid, 0), s_of_t, :]
        x[~valid] = 0.0
        xn = np.einsum("ij,ij->i", x, x).astype(np.float32)
        xn[~valid] = 1.0e4                      # forces K=0 on pad points
        xt = np.ascontiguousarray(x.T)          # [256, NPTS]
        aug = np.ascontiguousarray(
            np.stack([xn, np.ones(NPTS, np.float32)]))
        ccounts = counts[c * GPC:(c + 1) * GPC].astype(np.float64)
        a = np.where(ccounts > 0, 1.0 / np.maximum(ccounts * S, 1), 0.0)
        in_maps.append({
            "xt": xt,
            "aug": aug,
            "wmat": wmat,
            "waug": waug,
            "asc": (16.0 * a).astype(np.float32).reshape(CB, 1),
            "av": a.astype(np.float32).reshape(CB, 1),
            "sel": sel,
            "selt": selt,
        })
    return in_maps, NPAD


def kernel(node_distributions, batch_idx, codebook, _trace=False, _trace_kwargs=None):
    in_maps, NPAD = _host_shard(node_distributions, batch_idx, codebook)
    nc = build_nc(NPAD)
    nc.finalize()   # run bacc passes (reg alloc, wait splitting) before pjrt
    res = run_bass_kernel_spmd(nc, in_maps, list(range(NCORES)),
                               trace=_trace, **(_trace_kwargs or {}))
    out = np.zeros((B, CB), np.float32)
    for c in range(NCORES):
        out[c * GPC:(c + 1) * GPC, :] = res.results[c]["out"]
    kernel._last_exec_time_ns = res.exec_time_ns
    return out


# revision 17
# speedup vs baseline: 1.8471x; 1.8471x over previous
"""Barycentric pooling (segmented Sinkhorn onto a 16x256 codebook) on 8 trn2 cores.

Strategy (data-parallel per the sharding hint): the host sorts nodes by graph,
pads each graph to a fixed node count, and assigns 16 graphs to each of the 8
cores.  Each core receives a hidden-major [256, NPTS] tensor plus a 2-row
augmentation (squared-norm row, ones row).  On device, per 512-point tile,
three accumulating float32r matmuls against the stationary (augmented) codebook
produce -cost/2 in PSUM [16, 512]; ScalarE exp(20*x) emits K = exp(-cost/eps)
in bf16, and a DMA xbar transpose lands it point-major [128, NPAD, 16] in SBUF.
Graph g of a core owns partitions [8g, 8g+8).  The 20 Sinkhorn iterations run
on-chip: VectorE multiply + halving-tree adds for the two K passes, TensorE for
the tiny cross-partition sums (graph-sum via a selection matrix, v-broadcast
rebuild).  Cores are fully independent; the host assembles the [128, 16] output.
"""

import numpy as np
import ml_dtypes

import concourse.bass as bass
import concourse.bacc as bacc
import concourse.mybir as mybir
from concourse import tile
from concourse.bass_utils import run_bass_kernel_spmd

B = 128          # graphs
CB = 16          # codebook size
HID = 256
DIST = 8
EPS = 0.1
ITERS = 20
NCORES = 8
GPC = B // NCORES  # graphs per core = 16

F32 = mybir.dt.float32
F32R = mybir.dt.float32r
BF16 = mybir.dt.bfloat16

# q-slots per stage-level DMA transpose (TQ tiles * 4 slots each)
TGRP = 32
# column index of point (p, q) within a core's xt:
#   i = (q//TGRP)*(TGRP*128) + 128*(q%TGRP) + p
# (hardware-verified: the DMA-transpose out AP [128, u, 16] takes logical
# transposed row l at (p = l % 128, u = l // 128))
XMAP = "B"


def _col_of(p, q):
    if XMAP == "A":
        return 512 * (q // 4) + 4 * p + (q % 4)
    return 512 * (q // 4) + 128 * (q % 4) + p


def _tree_fold(nc, ap3, n):
    """Halving-tree add over the middle axis of ap3 [128, n, X] (in place,
    result in [:, 0, :]).  Handles odd n."""
    while n > 1:
        k = n // 2
        nc.vector.tensor_tensor(ap3[:, 0:k, :], ap3[:, 0:k, :],
                                ap3[:, n - k:n, :], mybir.AluOpType.add)
        n = n - k


def build_nc(NPAD: int, TQ: int = 8):
    """NPAD = padded nodes per graph (= points per partition); NPAD % 32 == 0."""
    NPTS = 128 * NPAD
    n_tiles = NPTS // 512          # 512-point tiles
    assert n_tiles % TQ == 0 and TGRP == 4 * TQ
    n_stages = n_tiles // TQ

    nc = bacc.Bacc(target_bir_lowering=False, debug=False)

    xt_ext = nc.declare_dram_parameter("xt", [HID, NPTS], BF16, isOutput=False)
    aug_ext = nc.declare_dram_parameter("aug", [2, NPTS], BF16, isOutput=False)
    wm_ext = nc.declare_dram_parameter("wmat", [HID, CB], BF16, isOutput=False)
    wa_ext = nc.declare_dram_parameter("waug", [2, CB], BF16, isOutput=False)
    ynb_ext = nc.declare_dram_parameter("ynb", [CB, 1], F32, isOutput=False)
    asc_ext = nc.declare_dram_parameter("asc", [CB, 1], F32, isOutput=False)
    av_ext = nc.declare_dram_parameter("av", [CB, 1], F32, isOutput=False)
    sel_ext = nc.declare_dram_parameter("sel", [128, CB], BF16, isOutput=False)
    selt_ext = nc.declare_dram_parameter("selt", [CB, 128], F32, isOutput=False)
    out_ext = nc.declare_dram_parameter("out", [CB, CB], F32, isOutput=True)

    with tile.TileContext(nc) as tc:
        with (
            tc.tile_pool(name="const", bufs=1) as cpool,
            tc.tile_pool(name="stage", bufs=3) as spool,
            tc.tile_pool(name="work", bufs=2) as wpool,
            tc.tile_pool(name="small", bufs=2) as mpool,
            tc.tile_pool(name="acc", bufs=4, space=bass.MemorySpace.PSUM) as apool,
            tc.tile_pool(name="psmall", bufs=2, space=bass.MemorySpace.PSUM) as ppool,
        ):
            # ---- constants ----
            wm_sb = cpool.tile([128, 2, CB], BF16, tag="wm")
            nc.sync.dma_start(wm_sb[:, 0, :], wm_ext[0:128, :])
            nc.sync.dma_start(wm_sb[:, 1, :], wm_ext[128:256, :])
            wa_sb = cpool.tile([2, CB], BF16, tag="wa")
            nc.sync.dma_start(wa_sb[:], wa_ext[:, :])
            ynb_sb = cpool.tile([CB, 1], F32, tag="ynb")
            nc.sync.dma_start(ynb_sb[:], ynb_ext[:, :])
            sel_sb = cpool.tile([128, CB], BF16, tag="sel")
            nc.sync.dma_start(sel_sb[:], sel_ext[:, :])
            selt_sb = cpool.tile([CB, 128], F32, tag="selt")
            nc.sync.dma_start(selt_sb[:], selt_ext[:, :])
            asc_sb = cpool.tile([CB, 1], F32, tag="asc")
            nc.sync.dma_start(asc_sb[:], asc_ext[:, :])
            av_sb = cpool.tile([CB, 1], F32, tag="av")
            nc.sync.dma_start(av_sb[:], av_ext[:, :])

            k_sb = cpool.tile([128, NPAD, CB], BF16, tag="K")
            v_sb = cpool.tile([CB, CB], F32, tag="v")
            nc.vector.memset(v_sb[:], 1.0)

            # ---- phase 1: K = exp(-cost/eps), via cb-major fp32r matmuls ----
            for st in range(n_stages):
                t0 = st * TQ
                cols = slice(t0 * 512, (t0 + TQ) * 512)
                xa0 = spool.tile([128, TQ * 512], BF16, tag="xa0")
                xa1 = spool.tile([128, TQ * 512], BF16, tag="xa1")
                xag = spool.tile([2, TQ * 512], BF16, tag="xag")
                nc.sync.dma_start(xa0[:], xt_ext[0:128, cols])
                nc.scalar.dma_start(xa1[:], xt_ext[128:256, cols])
                nc.sync.dma_start(xag[:], aug_ext[:, cols])
                kcb = spool.tile([CB, TQ * 512], BF16, tag="kcb")
                for tl in range(TQ):
                    pts = slice(tl * 512, (tl + 1) * 512)
                    acc = apool.tile([CB, 512], F32, tag="acc")
                    nc.tensor.matmul(acc[:], wm_sb[:, 0, :], xa0[:, pts],
                                     start=True, stop=False)
                    nc.tensor.matmul(acc[:], wm_sb[:, 1, :], xa1[:, pts],
                                     start=False, stop=False)
                    nc.tensor.matmul(acc[:], wa_sb[:], xag[:, pts],
                                     start=False, stop=True)
                    # K = exp(20 * (x.c - xn/2 - yn/2) ) = exp(-cost/0.1)
                    # (yn enters exactly via the f32 per-partition bias)
                    nc.scalar.activation(kcb[:, pts], acc[:],
                                         mybir.ActivationFunctionType.Exp,
                                         bias=ynb_sb[:],
                                         scale=20.0)
                # one batched xbar transpose per stage: [16, TQ*512] ->
                # [128, TGRP, 16] at q-slots [TGRP*st, TGRP*(st+1))
                nc.scalar.dma_start_transpose(
                    k_sb[:, TGRP * st:TGRP * (st + 1), :], kcb[:])

            # ---- phase 2: 20 Sinkhorn iterations ----
            for it in range(ITERS):
                # vrep[p, j] = v[p//8, j]
                vrep = ppool.tile([128, CB], F32, tag="vrep")
                nc.tensor.matmul(vrep[:], selt_sb[:], v_sb[:],
                                 start=True, stop=True)
                vrep_bf = mpool.tile([128, CB], BF16, tag="vrepbf")
                nc.vector.tensor_copy(vrep_bf[:], vrep[:])
                # d[p, q] = sum_j K[p,q,j] * v[g(p), j]
                prod = wpool.tile([128, NPAD, CB], BF16, tag="prod")
                nc.vector.tensor_tensor(
                    prod[:], k_sb[:],
                    vrep_bf[:].unsqueeze(1).broadcast_to([128, NPAD, CB]),
                    mybir.AluOpType.mult)
                # fold j: [128, NPAD, 16] -> [128, NPAD, 1]
                n = CB
                while n > 1:
                    k = n // 2
                    nc.vector.tensor_tensor(prod[:, :, 0:k], prod[:, :, 0:k],
                                            prod[:, :, n - k:n],
                                            mybir.AluOpType.add)
                    n = n - k
                # w = 1 / (d + 1e-8)
                d_sb = mpool.tile([128, NPAD], F32, tag="d")
                nc.vector.tensor_scalar_add(d_sb[:], prod[:, :, 0], 1e-8)
                w_sb = mpool.tile([128, NPAD], F32, tag="w")
                nc.vector.reciprocal(w_sb[:], d_sb[:])
                w_bf = mpool.tile([128, NPAD], BF16, tag="wbf")
                nc.vector.tensor_copy(w_bf[:], w_sb[:])
                # sp[p, j] = sum_q K[p,q,j] * w[p,q]
                prods = wpool.tile([128, NPAD, CB], BF16, tag="prod")
                nc.vector.tensor_tensor(
                    prods[:], k_sb[:],
                    w_bf[:].unsqueeze(2).broadcast_to([128, NPAD, CB]),
                    mybir.AluOpType.mult)
                _tree_fold(nc, prods, NPAD)
                # s_raw[k, j] = sum_{p in graph k} sp[p, j]
                sraw = ppool.tile([CB, CB], F32, tag="sraw")
                nc.tensor.matmul(sraw[:], sel_sb[:], prods[:, 0, :],
                                 start=True, stop=True)
                if it < ITERS - 1:
                    # v = 1 / (16*a*s_raw + 16e-8)
                    vden = mpool.tile([CB, CB], F32, tag="vden")
                    nc.vector.tensor_scalar(vden[:], sraw[:], asc_sb[:], 1.6e-7,
                                            mybir.AluOpType.mult,
                                            mybir.AluOpType.add)
                    nc.vector.reciprocal(v_sb[:], vden[:])
                else:
                    # final: tsum = a*s_raw*v ; weights = tsum / max(sum_j tsum, tiny)
                    t1 = mpool.tile([CB, CB], F32, tag="vden")
                    nc.vector.tensor_scalar(t1[:], sraw[:], av_sb[:], None,
                                            mybir.AluOpType.mult)
                    t2 = mpool.tile([CB, CB], F32, tag="t2")
                    nc.vector.tensor_tensor(t2[:], t1[:], v_sb[:],
                                            mybir.AluOpType.mult)
                    den = mpool.tile([CB, 1], F32, tag="den")
                    nc.vector.tensor_reduce(den[:], t2[:],
                                            mybir.AxisListType.X,
                                            mybir.AluOpType.add)
                    nc.vector.tensor_scalar(den[:], den[:], 1e-30, None,
                                            mybir.AluOpType.max)
                    rden = mpool.tile([CB, 1], F32, tag="rden")
                    nc.vector.reciprocal(rden[:], den[:])
                    outw = mpool.tile([CB, CB], F32, tag="outw")
                    nc.vector.tensor_scalar(outw[:], t2[:], rden[:], None,
                                            mybir.AluOpType.mult)
                    nc.sync.dma_start(out_ext[:, :], outw[:])

    return nc


def _host_shard(node_distributions, batch_idx, codebook):
    nd = np.ascontiguousarray(np.asarray(node_distributions, dtype=np.float32))
    bi = np.asarray(batch_idx).astype(np.int64).ravel()
    cb = np.asarray(codebook, dtype=np.float32)
    N, S, D = nd.shape
    assert S == DIST and D == HID

    counts = np.bincount(bi, minlength=B)[:B]
    NPAD = int(np.ceil(max(int(counts.max()), 64) / 64.0) * 64)
    NPTS = 128 * NPAD

    order = np.argsort(bi, kind="stable")
    slot = np.full((B, NPAD), -1, dtype=np.int64)
    mask = np.arange(NPAD)[None, :] < counts[:, None]
    slot[mask] = order

    # point (p, q): graph slot k = p//8; m = (p%8)*NPAD + q; node = m//S; s = m%S
    i = np.arange(NPTS)
    # invert the column map: recover (p, q) for each column i
    g, r = i // (TGRP * 128), i % (TGRP * 128)
    p = r % 128
    q = TGRP * g + r // 128
    k_of_i = p // 8
    m = (p % 8) * NPAD + q
    nl_of_i = m // S
    s_of_i = m % S

    yn = (cb * cb).sum(1)
    wmat = np.ascontiguousarray(cb.T).astype(ml_dtypes.bfloat16)   # [256, 16]
    # both aug rows (xn_hi, xn_lo) carry weight -0.5; yn goes in the exp bias
    waug = np.full((2, CB), -0.5, ml_dtypes.bfloat16)
    ynb = (-10.0 * yn).astype(np.float32).reshape(CB, 1)
    sel = np.zeros((128, CB), np.float32)
    sel[np.arange(128), np.arange(128) // 8] = 1.0
    selt = np.ascontiguousarray(sel.T)

    in_maps = []
    for c in range(NCORES):
        g = c * GPC + k_of_i                    # global graph per column
        nid = slot[g, nl_of_i]                  # node id or -1
        valid = nid >= 0
        x = nd[np.where(valid, nid, 0), s_of_i, :]
        x[~valid] = 0.0
        xn = np.einsum("ij,ij->i", x, x).astype(np.float32)
        xn[~valid] = 1.0e4                      # forces K=0 on pad points
        xt = np.ascontiguousarray(x.T).astype(ml_dtypes.bfloat16)  # [256, NPTS]
        xn_hi = xn.astype(ml_dtypes.bfloat16)
        xn_lo = (xn - xn_hi.astype(np.float32)).astype(ml_dtypes.bfloat16)
        aug = np.ascontiguousarray(np.stack([xn_hi, xn_lo]))
        ccounts = counts[c * GPC:(c + 1) * GPC].astype(np.float64)
        a = np.where(ccounts > 0, 1.0 / np.maximum(ccounts * S, 1), 0.0)
        in_maps.append({
            "xt": xt,
            "aug": aug,
            "wmat": wmat,
            "waug": waug,
            "ynb": ynb,
            "asc": (16.0 * a).astype(np.float32).reshape(CB, 1),
            "av": a.astype(np.float32).reshape(CB, 1),
            "sel": sel.astype(ml_dtypes.bfloat16),
            "selt": selt,
        })
    return in_maps, NPAD


def kernel(node_distributions, batch_idx, codebook, _trace=False, _trace_kwargs=None):
    in_maps, NPAD = _host_shard(node_distributions, batch_idx, codebook)
    nc = build_nc(NPAD)
    nc.finalize()   # run bacc passes (reg alloc, wait splitting) before pjrt
    res = run_bass_kernel_spmd(nc, in_maps, list(range(NCORES)),
                               trace=_trace, **(_trace_kwargs or {}))
    out = np.zeros((B, CB), np.float32)
    for c in range(NCORES):
        out[c * GPC:(c + 1) * GPC, :] = res.results[c]["out"]
    kernel._last_exec_time_ns = res.exec_time_ns
    kernel._last_res = res
    return out
